# revision 1
# baseline (speedup 1.0000x reference)
# Trainium2 Bass kernel for nn_BinLinearEval:
#   out[b, o] = (round(x @ W.T + bias) * sign >= 0) ? 1.0 : 0.0
#
# Math folding (exact because bias is integer-valued and sign in {-1,+1}):
#   out = 1  iff  sign*(dot + bias) >= -0.5
#       = 1  iff  dot' >= thr_o      where dot' = x @ (sign.T*W).T  (W' still
#         ternary, exact in fp16) and thr_o = -sign_o*bias_o - 0.5.
# The device computes dot' in two accumulated passes — an fp16 hi pass plus
# an fp8-e4m3 DoubleRow residual pass (x_lo*2^6 vs W'*2^-6, both exactly
# representable; DoubleRow contracts K=256 per matmul at ~1.75x the fp16
# rate) — giving near-fp32 accuracy (20/16.7M threshold flips) at ~60% of
# the 2xfp16 cost. Epilogue is a single per-partition is_ge threshold.
#
# Sharding: data-parallel over batch, 8192 rows per core. x is pre-transposed
# on the host to [feature, batch] layout so the contract dim lands on SBUF
# partitions; output is produced as [out, batch] per core and re-assembled /
# transposed on the host.

import os
from contextlib import ExitStack

import numpy as np
import ml_dtypes

BATCH, IN_F, OUT_F = 65536, 1024, 256
N_CORES = 8
B_CORE = BATCH // N_CORES  # 8192
P = 128
KC = IN_F // P             # 8 k-chunks
OC = OUT_F // P            # 2 out-channel chunks
BT = 512                   # matmul moving free dim
# Uniform small groups + deep buffering: DMA stays saturated and the PE
# never outruns the prefetch pipeline by more than the buffer depth.
GROUPS = [512] * (B_CORE // 512)
assert sum(GROUPS) == B_CORE
IO_BUFS = 6

_CACHE = {}


def _build():
    """Build (and cache) the Bass module. Returns the compiled nc."""
    if "nc" in _CACHE:
        return _CACHE["nc"]

    import concourse.bacc as bacc
    import concourse.mybir as mybir
    import concourse.tile as tile

    nc = bacc.Bacc(
        "TRN2",
        target_bir_lowering=False,
        debug=False,
        num_devices=N_CORES,
    )

    f16 = mybir.dt.float16
    f32 = mybir.dt.float32
    bf16 = mybir.dt.bfloat16
    f8 = mybir.dt.float8e4

    # group-major layouts: one group's slab is contiguous per partition
    # (8 KB / 4 KB descriptors instead of 1 KB / 512 B strided rows)
    n_groups = len(GROUPS)
    xhi_d = nc.dram_tensor(
        "xhi", [P, n_groups, KC, GROUPS[0]], f16, kind="ExternalInput"
    ).ap()
    xlo_d = nc.dram_tensor(
        "xlo8", [P, n_groups, KC // 2, 2, GROUPS[0]], f8, kind="ExternalInput"
    ).ap()
    wt_d = nc.dram_tensor("wt", [P, KC, OUT_F], f16, kind="ExternalInput").ap()
    wlo_d = nc.dram_tensor(
        "wlo8", [P, KC // 2, 2, OUT_F], f8, kind="ExternalInput"
    ).ap()
    thr_d = nc.dram_tensor("thr", [P, OC], f32, kind="ExternalInput").ap()
    out_d = nc.dram_tensor("out", [OC, P, B_CORE], bf16, kind="ExternalOutput").ap()

    with tile.TileContext(nc) as tc, ExitStack() as ctx:
        const = ctx.enter_context(tc.tile_pool(name="const", bufs=1))
        io = ctx.enter_context(tc.tile_pool(name="io", bufs=IO_BUFS))
        outp = ctx.enter_context(tc.tile_pool(name="outp", bufs=4))
        psum = ctx.enter_context(tc.tile_pool(name="psum", bufs=4, space="PSUM"))

        # consts ride the ACT HWDGE ring so the SP ring can start streaming
        # the first x group immediately; first matmul waits on whichever
        # finishes later (~2.8us instead of ~4.9us serialized)
        wt_sb = const.tile([P, KC, OUT_F], f16)
        nc.scalar.dma_start(out=wt_sb, in_=wt_d)
        wlo_sb = const.tile([P, KC // 2, 2, OUT_F], f8)
        nc.scalar.dma_start(out=wlo_sb, in_=wlo_d)
        thr_sb = const.tile([P, OC], f32)
        nc.scalar.dma_start(out=thr_sb, in_=thr_d)

        g0 = 0
        for g, group in enumerate(GROUPS):
            if g == 0:
                # split group 0's hi DMA by k-halves: the first matmuls gate
                # on 0.5 MB (+ completion receipt) instead of 1 MB, starting
                # the PE a few us earlier (quarters tested worse: per-DMA
                # completion receipts serialize)
                xh0a = io.tile(
                    [P, KC // 2, max(GROUPS)], f16, name="xh0a", bufs=1
                )
                xh0b = io.tile(
                    [P, KC // 2, max(GROUPS)], f16, name="xh0b", bufs=1
                )
                nc.sync.dma_start(out=xh0a, in_=xhi_d[:, 0, : KC // 2])
                nc.sync.dma_start(out=xh0b, in_=xhi_d[:, 0, KC // 2 :])

                def hi_ap(k, lo_, hi_):
                    t = xh0a if k < KC // 2 else xh0b
                    return t[:, k % (KC // 2), lo_:hi_]
            else:
                xhi_sb = io.tile([P, KC, max(GROUPS)], f16, name="xhi_sb")[
                    :, :, :group
                ]
                nc.sync.dma_start(out=xhi_sb, in_=xhi_d[:, g])

                def hi_ap(k, lo_, hi_, t=xhi_sb):
                    return t[:, k, lo_:hi_]

            xlo_sb = io.tile([P, KC // 2, 2, max(GROUPS)], f8, name="xlo_sb")[
                :, :, :, :group
            ]
            nc.sync.dma_start(out=xlo_sb, in_=xlo_d[:, g])
            for bt in range(group // BT):
                b0 = bt * BT
                for oc in range(OC):
                    ps = psum.tile([P, BT], f32, name="ps")
                    # all-hi then all-lo: the first matmuls of the kernel
                    # only need the hi half of the first group in SBUF
                    for k in range(KC):
                        nc.tensor.matmul(
                            ps,
                            wt_sb[:, k, oc * P : (oc + 1) * P],
                            hi_ap(k, b0, b0 + BT),
                            start=(k == 0),
                            stop=False,
                        )
                    # lo pass: fp8 e4m3 DoubleRow, contracts 256 per matmul
                    for c in range(KC // 2):
                        nc.tensor.matmul(
                            ps,
                            wlo_sb[:, c, :, oc * P : (oc + 1) * P],
                            xlo_sb[:, c, :, b0 : b0 + BT],
                            start=False,
                            stop=(c == KC // 2 - 1),
                            perf_mode=mybir.MatmulPerfMode.DoubleRow,
                        )
                    ob = outp.tile([P, BT], bf16, name="ob")
                    nc.vector.tensor_scalar(
                        ob,
                        ps,
                        thr_sb[:, oc : oc + 1],
                        None,
                        mybir.AluOpType.is_ge,
                    )
                    # out-DMAs ride the ACT HWDGE ring so they never block
                    # the input-DMA FIFO on the SP ring
                    nc.scalar.dma_start(
                        out=out_d[oc, :, g0 + b0 : g0 + b0 + BT], in_=ob
                    )
            g0 += group

    nc.compile()
    _CACHE["nc"] = nc
    return nc


def _prep_inputs(x, weight, bias, sign):
    """Host-side prep: fold sign into weights, build thresholds, split x into
    fp16 hi/lo, transpose to [feature, batch] per-core tiles."""
    x = np.asarray(x, dtype=np.float32)
    weight = np.asarray(weight, dtype=np.float32)
    bias = np.asarray(bias, dtype=np.float32)
    sign = np.asarray(sign, dtype=np.float32).reshape(1, OUT_F)

    wp = sign.T * weight                      # [OUT_F, IN_F], ternary
    thr = (-sign[0] * bias - np.float32(0.5)).astype(np.float32)  # [OUT_F]

    wt = np.ascontiguousarray(
        wp.T.reshape(KC, P, OUT_F).transpose(1, 0, 2)
    ).astype(np.float16)                      # [P, KC, OUT_F]
    thr2 = np.ascontiguousarray(thr.reshape(OC, P).T)  # [P, OC]

    xhi = x.astype(np.float16)
    f8np = ml_dtypes.float8_e4m3fn
    xlo8 = ((x - xhi.astype(np.float32)) * np.float32(64.0)).astype(f8np)
    wlo8 = np.ascontiguousarray(
        (wp.T * np.float32(1.0 / 64.0))
        .reshape(KC // 2, 2, P, OUT_F)
        .transpose(2, 0, 1, 3)
    ).astype(f8np)                            # [P, KC//2, 2, OUT_F]

    n_groups = len(GROUPS)
    grp = GROUPS[0]
    in_maps = []
    for c in range(N_CORES):
        sl = slice(c * B_CORE, (c + 1) * B_CORE)
        hi = np.ascontiguousarray(
            xhi[sl].reshape(n_groups, grp, KC, P).transpose(3, 0, 2, 1)
        )                                      # [P, n_groups, KC, grp]
        lo = np.ascontiguousarray(
            xlo8[sl]
            .reshape(n_groups, grp, KC // 2, 2, P)
            .transpose(4, 0, 2, 3, 1)
        )                                      # [P, n_groups, KC//2, 2, grp]
        in_maps.append(
            {"xhi": hi, "xlo8": lo, "wt": wt, "wlo8": wlo8, "thr": thr2}
        )
    return in_maps


def _assemble(results):
    """[core][OC, P, B_CORE] bf16 -> [BATCH, OUT_F] fp32"""
    full = np.concatenate(
        [r["out"].reshape(OUT_F, B_CORE) for r in results], axis=1
    )  # [OUT_F, BATCH]
    return np.ascontiguousarray(full.T).astype(np.float32)


def run(x, weight, bias, sign, trace=False):
    """Run the kernel; returns (output, BassKernelResults)."""
    from concourse.bass_utils import run_bass_kernel_spmd

    if not trace:
        # The NTFF profile hook module may be absent in this image; make
        # sure a stray BASS_TRACE=1 can't route us into the trace path.
        os.environ["BASS_NEVER_TRACE"] = "1"
    else:
        os.environ.pop("BASS_NEVER_TRACE", None)

    nc = _build()
    in_maps = _prep_inputs(x, weight, bias, sign)
    res = run_bass_kernel_spmd(
        nc,
        in_maps,
        core_ids=list(range(N_CORES)),
        trace=trace,
    )
    return _assemble(res.results), res


def kernel(x, weight, bias, sign):
    out, _ = run(x, weight, bias, sign, trace=False)
    return out



# revision 2
# speedup vs baseline: 1.1920x; 1.1920x over previous
# Trainium2 Bass kernel for nn_BinLinearEval:
#   out[b, o] = (round(x @ W.T + bias) * sign >= 0) ? 1.0 : 0.0
#
# Math folding (exact because bias is integer-valued and sign in {-1,+1}):
#   out = 1  iff  sign*(dot + bias) >= -0.5
#       = 1  iff  dot' >= thr_o      where dot' = x @ (sign.T*W).T  (W' still
#         ternary, exact in fp16) and thr_o = -sign_o*bias_o - 0.5.
# The device computes dot' in a single fp16 pass. fp16 x quantization gives
# ~560 threshold flips out of 16.7M (rel err ~0.008, 2.4x under the 2e-2
# gate) while halving both HBM traffic and PE time vs an hi+lo split.
# Epilogue is a single per-partition is_ge threshold emitted as fp8 (1.0/0.0
# are exact in e4m3), quartering output traffic vs bf16.
#
# Sharding: data-parallel over batch, 8192 rows per core. x is pre-transposed
# on the host to [feature, batch] layout so the contract dim lands on SBUF
# partitions; output is produced as [out, batch] per core and re-assembled /
# transposed on the host.
#
# DMA: x groups are split across BOTH HWDGE rings (SP + ACT) to lift the
# input stream above the single-queue ~290 GB/s ceiling; group DMAs are
# issued with a lookahead of L groups so an out-DMA waiting on the DVE
# epilogue can never head-of-line-block the next x prefetch on the ACT ring.

import os
from contextlib import ExitStack

import numpy as np
import ml_dtypes

BATCH, IN_F, OUT_F = 65536, 1024, 256
N_CORES = 8
B_CORE = BATCH // N_CORES  # 8192
P = 128
KC = IN_F // P             # 8 k-chunks
OC = OUT_F // P            # 2 out-channel chunks
GRP = 512                  # batch tile / group size (= max matmul moving dim)
N_GROUPS = B_CORE // GRP   # 16
LOOKAHEAD = 4
# ring split: 9 groups on the SP ring, 7 on the ACT ring (ACT also carries
# weights up front and 2 MB of out-DMAs throughout) -> ~9.0 vs ~9.5 MB
SYNC_GROUPS = frozenset([0, 2, 4, 6, 8, 10, 12, 14, 15])

_CACHE = {}


def _build():
    """Build (and cache) the Bass module. Returns the compiled nc."""
    if "nc" in _CACHE:
        return _CACHE["nc"]

    import concourse.bacc as bacc
    import concourse.mybir as mybir
    import concourse.tile as tile

    nc = bacc.Bacc(
        "TRN2",
        target_bir_lowering=False,
        debug=False,
        num_devices=N_CORES,
    )

    f16 = mybir.dt.float16
    f32 = mybir.dt.float32
    f8 = mybir.dt.float8e4

    xhi_d = nc.dram_tensor(
        "xhi", [P, N_GROUPS, KC, GRP], f16, kind="ExternalInput"
    ).ap()
    wt_d = nc.dram_tensor("wt", [P, KC, OUT_F], f16, kind="ExternalInput").ap()
    thr_d = nc.dram_tensor("thr", [P, OC], f32, kind="ExternalInput").ap()
    out_d = nc.dram_tensor("out", [OC, P, B_CORE], f8, kind="ExternalOutput").ap()

    with tile.TileContext(nc) as tc, ExitStack() as ctx:
        const = ctx.enter_context(tc.tile_pool(name="const", bufs=1))
        io = ctx.enter_context(tc.tile_pool(name="io", bufs=LOOKAHEAD + 2))
        outp = ctx.enter_context(tc.tile_pool(name="outp", bufs=4))
        psum = ctx.enter_context(tc.tile_pool(name="psum", bufs=4, space="PSUM"))

        # consts ride the ACT HWDGE ring so the SP ring can start streaming
        # the first x group immediately; first matmul waits on whichever
        # finishes later
        wt_sb = const.tile([P, KC, OUT_F], f16)
        nc.scalar.dma_start(out=wt_sb, in_=wt_d)
        thr_sb = const.tile([P, OC], f32)
        nc.scalar.dma_start(out=thr_sb, in_=thr_d)

        tiles = {}

        def issue(g):
            if g >= N_GROUPS:
                return
            if g == 0:
                # split group 0's DMA by k-halves: the first matmuls gate on
                # 0.5 MB (+ completion receipt) instead of 1 MB, starting the
                # PE earlier (quarters tested worse: per-DMA completion
                # receipts serialize)
                xh0a = io.tile([P, KC // 2, GRP], f16, name="xh0a", bufs=1)
                xh0b = io.tile([P, KC // 2, GRP], f16, name="xh0b", bufs=1)
                nc.sync.dma_start(out=xh0a, in_=xhi_d[:, 0, : KC // 2])
                nc.sync.dma_start(out=xh0b, in_=xhi_d[:, 0, KC // 2 :])

                def ap0(k, t0=xh0a, t1=xh0b):
                    t = t0 if k < KC // 2 else t1
                    return t[:, k % (KC // 2)]

                tiles[0] = ap0
            else:
                eng = nc.sync if g in SYNC_GROUPS else nc.scalar
                t = io.tile([P, KC, GRP], f16, name="xg")
                eng.dma_start(out=t, in_=xhi_d[:, g])

                def apg(k, t=t):
                    return t[:, k]

                tiles[g] = apg

        for g in range(LOOKAHEAD):
            issue(g)

        for g in range(N_GROUPS):
            issue(g + LOOKAHEAD)
            x_ap = tiles.pop(g)
            for oc in range(OC):
                ps = psum.tile([P, GRP], f32, name="ps")
                for k in range(KC):
                    nc.tensor.matmul(
                        ps,
                        wt_sb[:, k, oc * P : (oc + 1) * P],
                        x_ap(k),
                        start=(k == 0),
                        stop=(k == KC - 1),
                    )
                ob = outp.tile([P, GRP], f8, name="ob")
                nc.vector.tensor_scalar(
                    ob,
                    ps,
                    thr_sb[:, oc : oc + 1],
                    None,
                    mybir.AluOpType.is_ge,
                )
                nc.scalar.dma_start(
                    out=out_d[oc, :, g * GRP : (g + 1) * GRP], in_=ob
                )

    nc.compile()
    _CACHE["nc"] = nc
    return nc


def _prep_inputs(x, weight, bias, sign):
    """Host-side prep: fold sign into weights, build thresholds, cast x to
    fp16, transpose to [feature, batch] per-core tiles."""
    x = np.asarray(x, dtype=np.float32)
    weight = np.asarray(weight, dtype=np.float32)
    bias = np.asarray(bias, dtype=np.float32)
    sign = np.asarray(sign, dtype=np.float32).reshape(1, OUT_F)

    wp = sign.T * weight                      # [OUT_F, IN_F], ternary
    thr = (-sign[0] * bias - np.float32(0.5)).astype(np.float32)  # [OUT_F]

    wt = np.ascontiguousarray(
        wp.T.reshape(KC, P, OUT_F).transpose(1, 0, 2)
    ).astype(np.float16)                      # [P, KC, OUT_F]
    thr2 = np.ascontiguousarray(thr.reshape(OC, P).T)  # [P, OC]

    xhi = x.astype(np.float16)

    in_maps = []
    for c in range(N_CORES):
        sl = slice(c * B_CORE, (c + 1) * B_CORE)
        hi = np.ascontiguousarray(
            xhi[sl].reshape(N_GROUPS, GRP, KC, P).transpose(3, 0, 2, 1)
        )                                      # [P, N_GROUPS, KC, GRP]
        in_maps.append({"xhi": hi, "wt": wt, "thr": thr2})
    return in_maps


def _assemble(results):
    """[core][OC, P, B_CORE] fp8 -> [BATCH, OUT_F] fp32"""
    full = np.concatenate(
        [
            np.asarray(r["out"])
            .view(ml_dtypes.float8_e4m3fn)
            .astype(np.float32)
            .reshape(OUT_F, B_CORE)
            for r in results
        ],
        axis=1,
    )  # [OUT_F, BATCH]
    return np.ascontiguousarray(full.T)


def run(x, weight, bias, sign, trace=False):
    """Run the kernel; returns (output, BassKernelResults)."""
    from concourse.bass_utils import run_bass_kernel_spmd

    if not trace:
        os.environ["BASS_NEVER_TRACE"] = "1"
    else:
        os.environ.pop("BASS_NEVER_TRACE", None)

    nc = _build()
    in_maps = _prep_inputs(x, weight, bias, sign)
    res = run_bass_kernel_spmd(
        nc,
        in_maps,
        core_ids=list(range(N_CORES)),
        trace=trace,
    )
    return _assemble(res.results), res


def kernel(x, weight, bias, sign):
    out, _ = run(x, weight, bias, sign, trace=False)
    return out


# revision 3
# speedup vs baseline: 1.2478x; 1.0468x over previous
# Trainium2 Bass kernel for nn_BinLinearEval:
#   out[b, o] = (round(x @ W.T + bias) * sign >= 0) ? 1.0 : 0.0
#
# Math folding (exact because bias is integer-valued and sign in {-1,+1}):
#   out = 1  iff  sign*(dot + bias) >= -0.5
#       = 1  iff  dot' >= thr_o      where dot' = x @ (sign.T*W).T  (W' still
#         ternary) and thr_o = -sign_o*bias_o - 0.5.
#
# Precision: x is shipped as an e4m3 hi + e4m3 residual*64 pair (2 B/elem,
# same HBM bytes as fp16) and BOTH passes run as fp8 DoubleRow matmuls at
# 0.5 cycles/column - the PE stream is ~2x faster than the fp16 single-pass
# variant, which measured clock-throttled to ~2 GHz under a dense fp16 MM
# stream. Accuracy: ~1713 threshold flips of 16.7M (rel err ~0.0143 vs the
# 2e-2 gate; verified in fp64 emulation and stable because inputs and the
# accumulation order are deterministic).
#
# PE schedule: groups of 512 batch columns; blocks of up to 4 groups share
# each DoubleRow LDWEIGHTS (256-col loads at ~213 ns would otherwise pace
# the stream); all 8 PSUM banks hold the block's accumulators. Block sizes
# ramp [1,1,2,4,...] so the first matmul gates on 0.5 MB of DMA, not 4 MB.
#
# DMA: x groups split across BOTH HWDGE rings (SP + ACT); weight tensor is
# DMA'd in hi/lo halves so the first matmul gates on 0.25 MB of weights.
# Output is the is_ge threshold emitted as fp8 (1.0/0.0 exact), 1 B/elem.

import os
from contextlib import ExitStack

import numpy as np
import ml_dtypes

BATCH, IN_F, OUT_F = 65536, 1024, 256
N_CORES = 8
B_CORE = BATCH // N_CORES  # 8192
P = 128
KC = IN_F // P             # 8 k-chunks of 128
NCH = KC                   # 8 DoubleRow chunk-steps: 4 hi + 4 lo, 256-contract each
OC = OUT_F // P            # 2 out-channel chunks
GRP = 512                  # batch tile / group size (= max matmul moving dim)
N_GROUPS = B_CORE // GRP   # 16
BLOCKS = [1, 1, 2, 4, 4, 4]
assert sum(BLOCKS) == N_GROUPS
# ring split: ~9 MB of x on the SP ring; ACT ring carries 7 MB of x plus
# weights up front and 2 MB of out-DMAs throughout
SYNC_GROUPS = frozenset([0, 2, 4, 6, 8, 10, 12, 14, 15])

_CACHE = {}


def _build():
    """Build (and cache) the Bass module. Returns the compiled nc."""
    if "nc" in _CACHE:
        return _CACHE["nc"]

    import concourse.bacc as bacc
    import concourse.mybir as mybir
    import concourse.tile as tile

    nc = bacc.Bacc(
        "TRN2",
        target_bir_lowering=False,
        debug=False,
        num_devices=N_CORES,
    )

    f32 = mybir.dt.float32
    f8 = mybir.dt.float8e4
    DR = mybir.MatmulPerfMode.DoubleRow

    # x8 chunk layout: [P, group, chunk(0:4 hi, 4:8 lo), j, GRP] where the
    # DoubleRow pair (chunk c, j) covers global k = (c%4)*256 + j*128 + p
    x8_d = nc.dram_tensor(
        "x8", [P, N_GROUPS, NCH, 2, GRP], f8, kind="ExternalInput"
    ).ap()
    w8_d = nc.dram_tensor("w8", [P, NCH, 2, OUT_F], f8, kind="ExternalInput").ap()
    thr_d = nc.dram_tensor("thr", [P, OC], f32, kind="ExternalInput").ap()
    out_d = nc.dram_tensor("out", [OC, P, B_CORE], f8, kind="ExternalOutput").ap()

    with tile.TileContext(nc) as tc, ExitStack() as ctx:
        const = ctx.enter_context(tc.tile_pool(name="const", bufs=1))
        io = ctx.enter_context(tc.tile_pool(name="io", bufs=8))
        outp = ctx.enter_context(tc.tile_pool(name="outp", bufs=4))
        psum = ctx.enter_context(tc.tile_pool(name="psum", bufs=8, space="PSUM"))

        # weights ride the ACT ring in hi/lo halves so the first chunk-steps
        # gate on 0.25 MB; x group 0 streams on the SP ring concurrently
        w8_sb = const.tile([P, NCH, 2, OUT_F], f8)
        nc.scalar.dma_start(out=w8_sb[:, : NCH // 2], in_=w8_d[:, : NCH // 2])
        nc.scalar.dma_start(out=w8_sb[:, NCH // 2 :], in_=w8_d[:, NCH // 2 :])
        thr_sb = const.tile([P, OC], f32)
        nc.scalar.dma_start(out=thr_sb, in_=thr_d)

        tiles = {}

        def issue(g):
            if g >= N_GROUPS:
                return
            if g == 0:
                # split group 0 by hi/lo halves: the first 4 chunk-steps
                # gate on 0.5 MB (+ completion receipt) instead of 1 MB
                t = io.tile([P, NCH, 2, GRP], f8, name="xg0", bufs=1)
                nc.sync.dma_start(out=t[:, : NCH // 2], in_=x8_d[:, 0, : NCH // 2])
                nc.sync.dma_start(out=t[:, NCH // 2 :], in_=x8_d[:, 0, NCH // 2 :])
            else:
                eng = nc.sync if g in SYNC_GROUPS else nc.scalar
                t = io.tile([P, NCH, 2, GRP], f8, name="xg")
                eng.dma_start(out=t, in_=x8_d[:, g])
            tiles[g] = t

        blocks = []
        g0 = 0
        for b in BLOCKS:
            blocks.append(list(range(g0, g0 + b)))
            g0 += b

        for g in blocks[0] + blocks[1]:
            issue(g)

        for bi, blk in enumerate(blocks):
            if bi + 1 < len(blocks):
                for g in blocks[bi + 1]:
                    if g not in tiles:
                        issue(g)
            for oc in range(OC):
                pss = [psum.tile([P, GRP], f32, name="ps") for _ in blk]
                for c in range(NCH):
                    lhsT = w8_sb[:, c, :, oc * P : (oc + 1) * P]
                    for j, g in enumerate(blk):
                        nc.tensor.matmul(
                            pss[j],
                            lhsT,
                            tiles[g][:, c],
                            start=(c == 0),
                            stop=(c == NCH - 1),
                            perf_mode=DR,
                        )
                for j, g in enumerate(blk):
                    ob = outp.tile([P, GRP], f8, name="ob")
                    nc.vector.tensor_scalar(
                        ob,
                        pss[j],
                        thr_sb[:, oc : oc + 1],
                        None,
                        mybir.AluOpType.is_ge,
                    )
                    nc.scalar.dma_start(
                        out=out_d[oc, :, g * GRP : (g + 1) * GRP], in_=ob
                    )
            for g in blk:
                tiles.pop(g)

    nc.compile()
    _CACHE["nc"] = nc
    return nc


def _prep_inputs(x, weight, bias, sign):
    """Host-side prep: fold sign into weights, build thresholds, split x into
    an e4m3 hi + e4m3 residual*64 pair in DoubleRow-interleaved layout."""
    f8np = ml_dtypes.float8_e4m3fn
    x = np.asarray(x, dtype=np.float32)
    weight = np.asarray(weight, dtype=np.float32)
    bias = np.asarray(bias, dtype=np.float32)
    sign = np.asarray(sign, dtype=np.float32).reshape(1, OUT_F)

    wp = sign.T * weight                      # [OUT_F, IN_F], ternary
    thr = (-sign[0] * bias - np.float32(0.5)).astype(np.float32)  # [OUT_F]
    thr2 = np.ascontiguousarray(thr.reshape(OC, P).T)  # [P, OC]

    # weights: [P, chunk, j, OUT_F]; chunks 0:4 = W' (ternary, exact in
    # e4m3), 4:8 = W'/64 (+-2^-6, exact in e4m3)
    wT = wp.T  # [IN_F, OUT_F]
    whi = wT.reshape(NCH // 2, 2, P, OUT_F).transpose(2, 0, 1, 3)
    wlo = (wT * np.float32(1.0 / 64.0)).reshape(NCH // 2, 2, P, OUT_F).transpose(
        2, 0, 1, 3
    )
    w8 = np.ascontiguousarray(
        np.concatenate([whi, wlo], axis=1)
    ).astype(f8np)                            # [P, NCH, 2, OUT_F]

    xhi8 = x.astype(f8np)
    xlo8 = ((x - xhi8.astype(np.float32)) * np.float32(64.0)).astype(f8np)

    in_maps = []
    for c in range(N_CORES):
        sl = slice(c * B_CORE, (c + 1) * B_CORE)
        hi = xhi8[sl].reshape(N_GROUPS, GRP, NCH // 2, 2, P).transpose(
            4, 0, 2, 3, 1
        )                                      # [P, g, 4, 2, GRP]
        lo = xlo8[sl].reshape(N_GROUPS, GRP, NCH // 2, 2, P).transpose(
            4, 0, 2, 3, 1
        )
        x8 = np.ascontiguousarray(np.concatenate([hi, lo], axis=2))
        in_maps.append({"x8": x8, "w8": w8, "thr": thr2})
    return in_maps


def _assemble(results):
    """[core][OC, P, B_CORE] fp8 -> [BATCH, OUT_F] fp32"""
    full = np.concatenate(
        [
            np.asarray(r["out"])
            .view(ml_dtypes.float8_e4m3fn)
            .astype(np.float32)
            .reshape(OUT_F, B_CORE)
            for r in results
        ],
        axis=1,
    )  # [OUT_F, BATCH]
    return np.ascontiguousarray(full.T)


def run(x, weight, bias, sign, trace=False):
    """Run the kernel; returns (output, BassKernelResults)."""
    from concourse.bass_utils import run_bass_kernel_spmd

    if not trace:
        os.environ["BASS_NEVER_TRACE"] = "1"
    else:
        os.environ.pop("BASS_NEVER_TRACE", None)

    nc = _build()
    in_maps = _prep_inputs(x, weight, bias, sign)
    res = run_bass_kernel_spmd(
        nc,
        in_maps,
        core_ids=list(range(N_CORES)),
        trace=trace,
    )
    return _assemble(res.results), res


def kernel(x, weight, bias, sign):
    out, _ = run(x, weight, bias, sign, trace=False)
    return out


# revision 4
# speedup vs baseline: 1.3716x; 1.0992x over previous
# Trainium2 Bass kernel for nn_BinLinearEval:
#   out[b, o] = (round(x @ W.T + bias) * sign >= 0) ? 1.0 : 0.0
#
# Math folding (exact because bias is integer-valued and sign in {-1,+1}):
#   out = 1  iff  sign*(dot + bias) >= -0.5
#       = 1  iff  dot' >= thr_o      where dot' = x @ (sign.T*W).T  (W' still
#         ternary) and thr_o = -sign_o*bias_o - 0.5.
#
# Precision: x is shipped as an e4m3 hi + e4m3 residual*64 pair (2 B/elem,
# same HBM bytes as fp16) and BOTH passes run as fp8 DoubleRow matmuls at
# 0.5 cycles/column - the PE stream is ~2x faster than the fp16 single-pass
# variant, which measured clock-throttled to ~2 GHz under a dense fp16 MM
# stream. Accuracy: ~1713 threshold flips of 16.7M (rel err ~0.0143 vs the
# 2e-2 gate; verified in fp64 emulation and stable because inputs and the
# accumulation order are deterministic).
#
# PE schedule: groups of 512 batch columns; blocks of up to 4 groups share
# each DoubleRow LDWEIGHTS (256-col loads at ~213 ns would otherwise pace
# the stream); all 8 PSUM banks hold the block's accumulators. Block sizes
# ramp [1,1,2,4,...] so the first matmul gates on 0.5 MB of DMA, not 4 MB.
#
# DMA: x groups split across BOTH HWDGE rings (SP + ACT); weight tensor is
# DMA'd in hi/lo halves so the first matmul gates on 0.25 MB of weights.
# Output is the is_ge threshold emitted as fp8 (1.0/0.0 exact), 1 B/elem.

import os
from contextlib import ExitStack

import numpy as np
import ml_dtypes

BATCH, IN_F, OUT_F = 65536, 1024, 256
N_CORES = 8
B_CORE = BATCH // N_CORES  # 8192
P = 128
KC = IN_F // P             # 8 k-chunks of 128
NCH = KC                   # 8 DoubleRow chunk-steps: 4 hi + 4 lo, 256-contract each
OC = OUT_F // P            # 2 out-channel chunks
GRP = 512                  # batch tile / group size (= max matmul moving dim)
N_GROUPS = B_CORE // GRP   # 16
BLOCKS = [1, 1, 2, 4, 4, 4]
assert sum(BLOCKS) == N_GROUPS
# ring split: ~9 MB of x on the SP ring; ACT ring carries 7 MB of x plus
# weights up front and 2 MB of out-DMAs throughout
SYNC_GROUPS = frozenset([0, 2, 4, 6, 8, 10, 12, 14, 15])

_CACHE = {}


def _build():
    """Build (and cache) the Bass module. Returns the compiled nc."""
    if "nc" in _CACHE:
        return _CACHE["nc"]

    import concourse.bacc as bacc
    import concourse.mybir as mybir
    import concourse.tile as tile

    nc = bacc.Bacc(
        "TRN2",
        target_bir_lowering=False,
        debug=False,
        num_devices=N_CORES,
    )

    f32 = mybir.dt.float32
    f8 = mybir.dt.float8e4
    DR = mybir.MatmulPerfMode.DoubleRow

    # x8 chunk layout: [P, group, chunk(0:4 hi, 4:8 lo), j, GRP] where the
    # DoubleRow pair (chunk c, j) covers global k = (c%4)*256 + j*128 + p
    x8_d = nc.dram_tensor(
        "x8", [P, N_GROUPS, NCH, 2, GRP], f8, kind="ExternalInput"
    ).ap()
    w8_d = nc.dram_tensor("w8", [P, NCH, 2, OUT_F], f8, kind="ExternalInput").ap()
    thr_d = nc.dram_tensor("thr", [P, OC], f32, kind="ExternalInput").ap()
    out_d = nc.dram_tensor("out", [OC, P, B_CORE], f8, kind="ExternalOutput").ap()

    with tile.TileContext(nc) as tc, ExitStack() as ctx:
        const = ctx.enter_context(tc.tile_pool(name="const", bufs=1))
        io = ctx.enter_context(tc.tile_pool(name="io", bufs=12))
        outp = ctx.enter_context(tc.tile_pool(name="outp", bufs=4))
        psum = ctx.enter_context(tc.tile_pool(name="psum", bufs=8, space="PSUM"))

        # split the critical startup DMAs across both rings: weights (hi
        # half first) on SP, group 0's hi half on ACT -- the first
        # chunk-steps gate on 0.25 MB + 0.5 MB arriving in parallel
        w8_sb = const.tile([P, NCH, 2, OUT_F], f8)
        thr_sb = const.tile([P, OC], f32)
        tiles = {}
        xg0 = io.tile([P, NCH, 2, GRP], f8, name="xg0", bufs=1)
        tiles[0] = xg0
        nc.sync.dma_start(out=w8_sb[:, : NCH // 2], in_=w8_d[:, : NCH // 2])
        nc.scalar.dma_start(out=xg0[:, : NCH // 2], in_=x8_d[:, 0, : NCH // 2])
        nc.sync.dma_start(out=w8_sb[:, NCH // 2 :], in_=w8_d[:, NCH // 2 :])
        nc.scalar.dma_start(out=xg0[:, NCH // 2 :], in_=x8_d[:, 0, NCH // 2 :])
        nc.sync.dma_start(out=thr_sb, in_=thr_d)

        def issue(g):
            if g >= N_GROUPS or g in tiles:
                return
            eng = nc.sync if g in SYNC_GROUPS else nc.scalar
            t = io.tile([P, NCH, 2, GRP], f8, name="xg")
            eng.dma_start(out=t, in_=x8_d[:, g])
            tiles[g] = t

        blocks = []
        g0 = 0
        for b in BLOCKS:
            blocks.append(list(range(g0, g0 + b)))
            g0 += b

        for g in blocks[0] + blocks[1] + blocks[2]:
            issue(g)

        for bi, blk in enumerate(blocks):
            # 2-block prefetch lookahead keeps both rings streaming even
            # while out-DMAs wait on their epilogues
            for bj in (bi + 1, bi + 2):
                if bj < len(blocks):
                    for g in blocks[bj]:
                        issue(g)
            for oc in range(OC):
                pss = [psum.tile([P, GRP], f32, name="ps") for _ in blk]
                for c in range(NCH):
                    lhsT = w8_sb[:, c, :, oc * P : (oc + 1) * P]
                    for j, g in enumerate(blk):
                        nc.tensor.matmul(
                            pss[j],
                            lhsT,
                            tiles[g][:, c],
                            start=(c == 0),
                            stop=(c == NCH - 1),
                            perf_mode=DR,
                        )
                # one fat out-DMA per (block, oc): 2 KB DRAM lines instead
                # of 512 B, and 4x fewer descriptors on the ACT ring
                ob = outp.tile([P, len(blk) * GRP], f8, name=f"ob{len(blk)}")
                for j, g in enumerate(blk):
                    nc.vector.tensor_scalar(
                        ob[:, j * GRP : (j + 1) * GRP],
                        pss[j],
                        thr_sb[:, oc : oc + 1],
                        None,
                        mybir.AluOpType.is_ge,
                    )
                nc.scalar.dma_start(
                    out=out_d[oc, :, blk[0] * GRP : (blk[-1] + 1) * GRP], in_=ob
                )
            for g in blk:
                tiles.pop(g)

    nc.compile()
    _CACHE["nc"] = nc
    return nc


def _prep_inputs(x, weight, bias, sign):
    """Host-side prep: fold sign into weights, build thresholds, split x into
    an e4m3 hi + e4m3 residual*64 pair in DoubleRow-interleaved layout."""
    f8np = ml_dtypes.float8_e4m3fn
    x = np.asarray(x, dtype=np.float32)
    weight = np.asarray(weight, dtype=np.float32)
    bias = np.asarray(bias, dtype=np.float32)
    sign = np.asarray(sign, dtype=np.float32).reshape(1, OUT_F)

    wp = sign.T * weight                      # [OUT_F, IN_F], ternary
    thr = (-sign[0] * bias - np.float32(0.5)).astype(np.float32)  # [OUT_F]
    thr2 = np.ascontiguousarray(thr.reshape(OC, P).T)  # [P, OC]

    # weights: [P, chunk, j, OUT_F]; chunks 0:4 = W' (ternary, exact in
    # e4m3), 4:8 = W'/64 (+-2^-6, exact in e4m3)
    wT = wp.T  # [IN_F, OUT_F]
    whi = wT.reshape(NCH // 2, 2, P, OUT_F).transpose(2, 0, 1, 3)
    wlo = (wT * np.float32(1.0 / 64.0)).reshape(NCH // 2, 2, P, OUT_F).transpose(
        2, 0, 1, 3
    )
    w8 = np.ascontiguousarray(
        np.concatenate([whi, wlo], axis=1)
    ).astype(f8np)                            # [P, NCH, 2, OUT_F]

    xhi8 = x.astype(f8np)
    xlo8 = ((x - xhi8.astype(np.float32)) * np.float32(64.0)).astype(f8np)

    in_maps = []
    for c in range(N_CORES):
        sl = slice(c * B_CORE, (c + 1) * B_CORE)
        hi = xhi8[sl].reshape(N_GROUPS, GRP, NCH // 2, 2, P).transpose(
            4, 0, 2, 3, 1
        )                                      # [P, g, 4, 2, GRP]
        lo = xlo8[sl].reshape(N_GROUPS, GRP, NCH // 2, 2, P).transpose(
            4, 0, 2, 3, 1
        )
        x8 = np.ascontiguousarray(np.concatenate([hi, lo], axis=2))
        in_maps.append({"x8": x8, "w8": w8, "thr": thr2})
    return in_maps


def _assemble(results):
    """[core][OC, P, B_CORE] fp8 -> [BATCH, OUT_F] fp32"""
    full = np.concatenate(
        [
            np.asarray(r["out"])
            .view(ml_dtypes.float8_e4m3fn)
            .astype(np.float32)
            .reshape(OUT_F, B_CORE)
            for r in results
        ],
        axis=1,
    )  # [OUT_F, BATCH]
    return np.ascontiguousarray(full.T)


def run(x, weight, bias, sign, trace=False):
    """Run the kernel; returns (output, BassKernelResults)."""
    from concourse.bass_utils import run_bass_kernel_spmd

    if not trace:
        os.environ["BASS_NEVER_TRACE"] = "1"
    else:
        os.environ.pop("BASS_NEVER_TRACE", None)

    nc = _build()
    in_maps = _prep_inputs(x, weight, bias, sign)
    res = run_bass_kernel_spmd(
        nc,
        in_maps,
        core_ids=list(range(N_CORES)),
        trace=trace,
    )
    return _assemble(res.results), res


def kernel(x, weight, bias, sign):
    out, _ = run(x, weight, bias, sign, trace=False)
    return out
